# revision 69
# baseline (speedup 1.0000x reference)
"""Multi-head attention (shared Wq for Q/K/V projections, Wo output proj)
as a Bass/Tile kernel for 8 Trainium2 NeuronCores.

Problem: B=4, S=2048, D=1024, H=16 heads (dk=64).
  Q = q @ Wq.T ; K = k @ Wq.T ; V = v @ Wq.T   (faithful: Wq for all three)
  out = softmax(Q K^T / 8) V  -> merge heads -> @ Wo.T

Sharding: core c handles batch b=c//2 and head-half half=c%2 (8 heads = 512
projection columns). Each core computes a partial (S, D) output
(head_out_slice @ Wo.T rows) in fp32; host sums the two halves per batch.

Per-core device pipeline (all matmuls fp16 — same PE rate as bf16 on TRN2
but 4x finer mantissa — with fp32 PSUM accumulation). Iterations run
pair-outer / query-block-inner so projection deadlines spread 4x wider and
the output projection lands in the last four iterations, where the
ACT-paced attention stream has PE slack.
  P1 projections: minimal head (KT[0] q0 + QT[0] q0); everything else —
     KT[0] q1-3, all 16 V tiles, remaining K/Q tiles — streams through the
     attention boundaries as deadline-tagged items pumped just-in-time
     (~1 iteration before first use). Greedy pumping instead runs the
     background queue dry by iteration ~10 and leaves the ACT-paced late
     iterations with no PE filler (~200ns st-tile wait per MM1 pack).
  P2 attention per (pair, query-block), per kc chunk of 128 keys:
       MM1: the two heads issued as a row-tiled pack (tile_position
            (0,0)/(64,0), K=64 each on disjoint row-groups) — both stream
            concurrently, ~306ns/pack vs 432ns serial (XBUS sustains
            ~1.7 col/cycle across the two streams; the pack's two
            LDWEIGHTS also consume both PE weight slots, exposing the
            next matmul's load). One exp group == one pack so both pack
            members share identical deps — the Tile scheduler splits
            packs whose members stall differently.
       ACT: PT = exp(ST * 1/8) PSUM->SBUF fp16, N=1024 per ACTIVATE
            (~1.04us; ACT paces the steady state at ~267us total).
       MM2: accum (65, 512) += V_aug (128 keys, 64+1 ones col) @ PT,
            trailing TDU=14 units behind exp (ptp bufs bound the lag); a
            deeper drain (flush cap 3) regresses ~70us — MM2 catches the
            exp stream and head-of-line blocks. Row 64 accumulates the
            softmax denominator for free.
       tail: evict accum to SBUF, reciprocal via DRAM bounce reshaped
             (32, 2, 16) (64B descriptors), DMA partition-broadcast,
             normalize -> head_outT fp16.
  P3 output projection per (query-block, qk chunk), gated >= 10 groups
     after the normalize chain starts; a few chunks are held back for the
     final drain so the last chain's ~6us latency is covered with ready
     PE work (keeps HAM at full clock through the tail). Output DMAs go
     on the gpsimd queue to keep the sync queue clear for staging and
     normalize-chain hops.
"""

import numpy as np
import ml_dtypes  # noqa: F401  (kept for environments keying on bf16 dtypes)

F16 = np.float16

S = 2048          # sequence length
D = 1024          # model dim
COLS = 512        # projection columns per core (8 heads * 64)
P = 128           # SBUF partitions
DK = 64           # head dim
PAIRS = 4         # head pairs per core
KC = S // P       # 16 key chunks
RC = D // P       # 8 contraction chunks for projections
QB = 512          # query block size
NQB = S // QB     # 4 query blocks
N_CORES = 8

_PROGRAM_CACHE = {}


def _emit_kernel(tc, aps):
    import concourse.mybir as mybir

    nc = tc.nc
    f32 = mybir.dt.float32
    bf16 = mybir.dt.float16  # fp16: same PE rate as bf16, 4x finer mantissa
    Exp = mybir.ActivationFunctionType.Exp
    mult = mybir.AluOpType.mult

    qT, kT, vT, wq, wo, out = (
        aps["qT"], aps["kT"], aps["vT"], aps["wq"], aps["wo"], aps["out"])

    # head_outT per (pair, qcb): (128 pair-dims, 512 q)
    HOUT = [[None] * NQB for _ in range(PAIRS)]

    with (
        tc.tile_pool(name="persist", bufs=1) as persist,
        tc.tile_pool(name="stage", bufs=4) as stage,        # 4 x 16KB/part
        tc.tile_pool(name="ptp", bufs=8) as ptp,            # exp outputs
        tc.tile_pool(name="hop", bufs=16) as hop,  # head_outT tiles
        tc.tile_pool(name="smalls", bufs=2) as smalls,
        tc.tile_pool(name="osbp", bufs=3) as osbp,
        tc.tile_pool(name="dramp", bufs=4, space="DRAM") as dramp,
        tc.tile_pool(name="stps", bufs=1, space="PSUM") as stps,
        tc.tile_pool(name="pbp", bufs=4, space="PSUM") as pbp,
    ):
        # ---------------- persistent SBUF tiles ----------------
        def ptile(shape, name):
            return persist.tile(shape, bf16, tag=name, name=name)

        wq_sb = ptile([P, RC, COLS], "wq_sb")                   # 8 KB/part
        wo_sb = ptile([P, PAIRS, D], "wo_sb")                   # 8 KB/part
        QT = [ptile([P, S], f"QT{p}") for p in range(PAIRS)]
        KT = [ptile([P, S], f"KT{p}") for p in range(PAIRS)]
        # V with ones column per head: (seq part, 8 heads, 64+1); the ones
        # column makes MM2 row 64 accumulate the softmax denominator free.
        V = [ptile([P, 8, DK + 1], f"V{kc}") for kc in range(KC)]

        wq_r = wq.rearrange("(r p) n -> p r n", p=P)
        nc.sync.dma_start(wq_sb[:, 0:2, :], wq_r[:, 0:2, :])
        nc.sync.dma_start(wq_sb[:, 2:RC, :], wq_r[:, 2:RC, :])

        # HAM warmup: ~16 dummy matmuls on a memset tile, dependent on no
        # DMA, keep the PE continuously busy from ~6us so the clock gate
        # flips to 2.4GHz at ~9us — the real projection stream otherwise
        # runs at half clock until ~20us waiting out the 3.4us busy window.
        # Reuses the stA PSUM slot (free until the first real exp group).
        warm = persist.tile([P, 640], bf16, tag="warm", name="warm")
        nc.vector.memset(warm[:], 0.0)
        wst = stps.tile([P, 2 * QB], f32, tag="stA", name="wst")
        for i in range(16):
            nc.tensor.matmul(
                wst[:, (i % 2) * QB:(i % 2 + 1) * QB],
                warm[:, 0:P], warm[:, P:P + QB], start=True, stop=True)

        # ---------------- P1: projections (chunked staging) ----------------
        # Stage (128, 8, 512) column-chunks of the transposed inputs on
        # demand. Only V, KT[0] and QT[0][qc0] are projected up front; the
        # remaining 27 projection tiles are deadline-ordered background work
        # pumped into the attention stream (PE has slack under the ACT-bound
        # exp pipeline).
        kch = [None] * NQB
        qch = [None] * NQB

        # kc/qc chunks are read by every pair across the whole run: they
        # must all stay cached (bufs == n chunks), or a buffer-rotation
        # wait on a later-pumped reader deadlocks. vc is consumed
        # sequentially during iteration 0, so 2 rotating bufs suffice.
        CHUNK_BUFS = {"kc": 4, "qc": 4, "vc": 2}

        def chunk(cache, src, qc, tagn, bufs=None):
            if cache[qc] is None:
                t = stage.tile([P, RC, QB], bf16, tag=tagn,
                               bufs=CHUNK_BUFS[tagn], name=f"{tagn}{qc}")
                srcv = src.rearrange("(r p) n -> p r n", p=P)[
                    :, :, qc * QB:(qc + 1) * QB]
                # split transfers: a projection's first rc-matmuls wait on
                # the first piece only (subtile deps). The head-critical
                # first chunks ship in rc-quarters for minimum first-matmul
                # latency.
                npc = 4 if qc == 0 and tagn in ("kc", "qc") else 2
                step = RC // npc
                for pc in range(npc):
                    nc.sync.dma_start(
                        t[:, pc * step:(pc + 1) * step, :],
                        srcv[:, pc * step:(pc + 1) * step, :])
                cache[qc] = t
            return cache[qc]

        def proj_psum(lhsT_of_rc, rhs_of_rc, n_free):
            ps = pbp.tile([P, QB], f32, tag="pb", name="projps")
            for rc in range(RC):
                nc.tensor.matmul(
                    ps[:, :n_free], lhsT_of_rc(rc), rhs_of_rc(rc),
                    start=(rc == 0), stop=(rc == RC - 1))
            return ps

        vch = [None] * NQB

        def proj_v_tile(kc):
            qc, k4 = divmod(kc, 4)
            vc = chunk(vch, vT, qc, "vc")
            ps = proj_psum(
                lambda rc, k4=k4: vc[:, rc, k4 * P:(k4 + 1) * P],
                lambda rc: wq_sb[:, rc, :],
                COLS)
            nc.vector.tensor_copy(
                out=V[kc][:, :, 0:DK],
                in_=ps.rearrange("p (h d) -> p h d", d=DK))
            nc.vector.memset(V[kc][:, :, DK:DK + 1], 1.0)
            v_done[0] += 1

        def proj_k(pair, qc):
            t = chunk(kch, kT, qc, "kc")
            ps = proj_psum(
                lambda rc: wq_sb[:, rc, pair * P:(pair + 1) * P],
                lambda rc: t[:, rc, :],
                QB)
            nc.vector.tensor_copy(
                out=KT[pair][:, qc * QB:(qc + 1) * QB], in_=ps[:])

        def proj_q(pair, qc):
            t = chunk(qch, qT, qc, "qc")
            ps = proj_psum(
                lambda rc: wq_sb[:, rc, pair * P:(pair + 1) * P],
                lambda rc: t[:, rc, :],
                QB)
            nc.vector.tensor_copy(
                out=QT[pair][:, qc * QB:(qc + 1) * QB], in_=ps[:])

        # Prefetch input chunks so the PE stream starts as soon as wq + the
        # first kT chunk land (HAM warms ~3.4us after the first dense matmul
        # burst; lazy staging left PE cold for 23us). Prefetch only within
        # each stage tag's buf count — a dma_start whose source buffer is
        # still owned by an unprojected chunk would stall the sync queue.
        # wo is not needed until the first output projection, so its DMA
        # queues after the prefetches.
        chunk(kch, kT, 0, "kc")
        chunk(qch, qT, 0, "qc")
        chunk(kch, kT, 1, "kc")
        chunk(vch, vT, 0, "vc")
        chunk(kch, kT, 2, "kc")
        chunk(vch, vT, 1, "vc")
        chunk(kch, kT, 3, "kc")
        chunk(qch, qT, 1, "qc")
        chunk(qch, qT, 2, "qc")
        chunk(qch, qT, 3, "qc")
        nc.sync.dma_start(wo_sb[:], wo.rearrange("(c p) n -> p c n", p=P))

        # Minimal head: KT[0] q-chunk 0 + QT[0] q-chunk 0 is all the first
        # attention kc-iterations need (MM1 consumes KT in kc-slices). The
        # rest of KT[0], all V tiles, and pair 1's first tiles stream
        # through iteration-0 boundaries as deadline-ordered items, so the
        # first exp fires ~10us in instead of ~50us.
        proj_k(0, 0)
        proj_q(0, 0)
        v_done = [0]

        # background projection items, ordered by first-use deadline.
        # NOTE: V must NOT go here — iteration i's own MM2 consumes V, so a
        # V-proj PSUM allocation that waits on iteration i's accumulator slot
        # release forms a dependency cycle (observed as first-exec NaN).
        # Each background projection tile is split into two 4-rc half-thunks
        # (sharing one PSUM tile via `box`) so a single pump injects ~850ns
        # of PE work instead of 1.8us — keeping ACT fed in the steady state.
        def make_proj_halves(kind, pair, qc):
            box = {}
            cache, src, tagn, dst = (
                (kch, kT, "kc", KT) if kind == "K" else (qch, qT, "qc", QT))

            def half(h):
                t = chunk(cache, src, qc, tagn)
                if h == 0:
                    box["ps"] = pbp.tile([P, QB], f32, tag="pb", name="projps")
                ps = box["ps"]
                for rc in range(h * 4, h * 4 + 4):
                    nc.tensor.matmul(
                        ps[:], wq_sb[:, rc, pair * P:(pair + 1) * P],
                        t[:, rc, :],
                        start=(rc == 0), stop=(rc == RC - 1))
                if h == 1:
                    nc.vector.tensor_copy(
                        out=dst[pair][:, qc * QB:(qc + 1) * QB], in_=ps[:])
            return [lambda: half(0), lambda: half(1)]

        # Deadline-ordered background projections for pair-outer iteration
        # order (iteration index = 4*pair + qcb): KT[p] is consumed in
        # kc-slices from iteration 4p, QT[p][qc] at iteration 4p+qc — all
        # deadlines sit far behind the pump rate, so the projection work
        # spreads thin instead of starving ACT in the first iterations.
        # Each entry: (deadline_iteration, halves). KT[p] must be complete
        # by iteration 4p (consumed in kc-slices from its start), QT[p][qc]
        # by iteration 4p+qc. Items are pumped just-in-time (~1.5
        # iterations early) rather than greedily: pumped greedily, bg runs
        # dry by iteration ~10 and every later MM1 pack waits ~200ns on its
        # st tile for the ACT-paced exp stream.
        bg_items = []
        for dl, it in [(2, "Q0q2"), (3, "Q0q3"), (4, "K1q0"), (4, "K1q1"),
                       (4, "K1q2"), (4, "K1q3"), (4, "Q1q0"), (5, "Q1q1"),
                       (6, "Q1q2"), (7, "Q1q3"), (8, "K2q0"), (8, "K2q1"),
                       (8, "K2q2"), (8, "K2q3"), (8, "Q2q0"), (9, "Q2q1"),
                       (10, "Q2q2"), (11, "Q2q3"), (12, "K3q0"),
                       (12, "K3q1"), (12, "K3q2"), (12, "K3q3"),
                       (12, "Q3q0"), (13, "Q3q1"), (14, "Q3q2"),
                       (15, "Q3q3")]:
            bg_items.append(
                (dl, make_proj_halves(it[0], int(it[1]), int(it[3]))))

        # Iteration-0 boundary items (up to two per boundary), deadline
        # ordered: KT[0]'s remaining q-chunks arrive before the attention
        # kc-loop reaches them (K0q_c by boundary 4c), V[kc] before the MM2
        # flush stream reaches kc (~boundary kc+9), and QT[0] q-chunk 1
        # before iteration 1.
        iter0_items = []
        _i0 = [("K", 0, 1), ("V", 0), ("V", 1), ("K", 0, 2), ("V", 2),
               ("V", 3), ("V", 4), ("K", 0, 3), ("V", 5), ("V", 6),
               ("Q", 0, 1), ("V", 7), ("V", 8), ("V", 9), ("V", 10),
               ("V", 11), ("V", 12), ("V", 13), ("V", 14), ("V", 15)]
        for it in _i0:
            if it[0] == "V":
                iter0_items.append(
                    [lambda kc=it[1]: proj_v_tile(kc)])
            else:
                iter0_items.append(make_proj_halves(it[0], it[1], it[2]))

        # ---------------- P2+P3: attention + output projection ----------------
        # Software pipeline: MM2 consumption trails MM1/exp production by TD
        # groups (globally, across iteration boundaries), so the in-order PE
        # stream never blocks on a just-issued exp. Output-projection work is
        # emitted in small chunks between groups to avoid starving ACT.
        TD = 4

        def emit_tail(pair, qcb, accum):
            # Evict accumulators to SBUF immediately: frees the PSUM slots.
            raw = [smalls.tile([DK + 1, QB], f32, tag="raw", bufs=4,
                               name=f"raw{j}") for j in (0, 1)]
            for j in (0, 1):
                nc.vector.tensor_copy(out=raw[j][:], in_=accum[j][:])
            # Normalize by softmax denominator (raw row 64), off critical
            # path. DVE reciprocal on a (1, 512) AP would use one lane
            # (~3.3us); bounce through DRAM and reload as (128, 2, 4) so all
            # 128 lanes divide in parallel.
            rdram = dramp.tile([2, QB], f32, name="rdram")
            for j in (0, 1):
                nc.sync.dma_start(rdram[j:j + 1, :], raw[j][DK:DK + 1, :])
            # Reload 32-wide x 16-deep: 64B descriptors (vs 16B at p=128)
            # quarter the descriptor count; recip on 32 lanes is still fast.
            rs = smalls.tile([32, 2, QB // 32], f32, tag="rs", name="rs")
            nc.sync.dma_start(rs[:], rdram.rearrange("j (p f) -> p j f", p=32))
            rr = smalls.tile([32, 2, QB // 32], f32, tag="rr", name="rr")
            nc.vector.reciprocal(rr[:], rs[:])
            rdram2 = dramp.tile([2, QB], f32, name="rdram2")
            nc.sync.dma_start(rdram2.rearrange("j (p f) -> p j f", p=32), rr[:])
            bcast = [smalls.tile([DK, QB], f32, tag="bcast", bufs=2,
                                 name=f"bcast{j}") for j in (0, 1)]
            for j in (0, 1):
                nc.sync.dma_start(
                    bcast[j][:], rdram2[j:j + 1, :].to_broadcast((DK, QB)))
            ht = hop.tile([P, QB], bf16, tag="hout", name=f"ht{pair}_{qcb}")
            for j in (0, 1):
                nc.vector.tensor_tensor(
                    ht[j * DK:(j + 1) * DK, :],
                    raw[j][0:DK, :],
                    bcast[j][:],
                    mult)
            HOUT[pair][qcb] = ht

        def outproj_qk(qcb, qk):
            q0 = qcb * QB
            osb = osbp.tile([P, D], f32, tag="osb", name="osb")
            for nk in range(2):
                ps = pbp.tile([P, QB], f32, tag="pb", name="ops")
                for pair in range(PAIRS):
                    nc.tensor.matmul(
                        ps[:],
                        HOUT[pair][qcb][:, qk * P:(qk + 1) * P],
                        wo_sb[:, pair, nk * QB:(nk + 1) * QB],
                        start=(pair == 0), stop=(pair == PAIRS - 1))
                nc.vector.tensor_copy(out=osb[:, nk * QB:(nk + 1) * QB], in_=ps[:])
                # Output DMA on the gpsimd queue: on sync it queues behind
                # the normalize-chain hops (and vice versa), and an osb
                # buffer whose free is stuck behind that backlog stalls
                # outproj at the tail. Issued per-nk half so the transfer
                # starts right after each eviction.
                nc.gpsimd.dma_start(
                    out[q0 + qk * P: q0 + (qk + 1) * P,
                        nk * QB:(nk + 1) * QB],
                    osb[:, nk * QB:(nk + 1) * QB])

        pending = []       # deferred MM2 units: (pair, qcb, pt, u, j, kc, last)
        iter_accum = {}    # (pair, qcb) -> [accum0, accum1]
        oproj_items = []   # deferred outproj chunks: (ready_at_gc, qcb, qk)
        held_items = []    # outproj chunks reserved for the final drain
        gc_box = [0]       # global group counter

        def pump(it_idx):
            # deadline-due bg first, then outproj; leave not-yet-due bg for
            # the iterations it was scheduled to fill
            if bg_items and bg_items[0][0] - 1 <= it_idx:
                for th in bg_items.pop(0)[1]:
                    th()
            elif oproj_items and oproj_items[0][0] <= gc_box[0]:
                _, oq, ok = oproj_items.pop(0)
                outproj_qk(oq, ok)

        def flush_mm2_unit():
            pair, qcb, pt, u, j, kc, last = pending.pop(0)
            key = (pair, qcb)
            if key not in iter_accum:
                iter_accum[key] = [
                    pbp.tile([DK + 1, QB], f32, tag="pb", name=f"acc{jj}")
                    for jj in (0, 1)]
            accum = iter_accum[key]
            nc.tensor.matmul(
                accum[j][:],
                V[kc][:, pair * 2 + j, :],
                pt[:, u * QB:(u + 1) * QB],
                start=(kc == 0), stop=(kc == KC - 1))
            if last:
                emit_tail(pair, qcb, accum)
                del iter_accum[key]
                if pair == PAIRS - 1:
                    # The normalize chain (reciprocal via DRAM bounce +
                    # broadcast) takes ~8us; don't let the in-order PE stream
                    # hit outproj matmuls before head_outT can possibly be
                    # ready, or the whole pipeline stalls head-of-line.
                    # qcb<3 hold back their last two qk chunks for the final
                    # drain: that ready-to-run work keeps the PE busy (and
                    # HAM warm) under the last iteration's normalize chain,
                    # which otherwise exposes ~12us of idle.
                    # With pair-outer order all outproj is late anyway; hold
                    # just enough chunks to cover the final normalize chain.
                    n_hold = {0: 1, 1: 2, 2: 2, 3: 0}[qcb]
                    for qk in range(QB // P):
                        if qk >= (QB // P) - n_hold:
                            held_items.append((qcb, qk))
                        else:
                            oproj_items.append(
                                (gc_box[0] + 10 + qk, qcb, qk))

        # MM2 trails MM1/exp by TDU units (7 exp-groups back; ptp has 8
        # bufs, so the exp stream keeps one pt tile of slack).
        TDU = 14

        # One exp group per kc: the group's two units are the row-tiled MM1
        # pack for that kc's head pair (tile_position (0,0) / (64,0), each
        # contracting its head's 64 dk rows on disjoint row-groups, so both
        # stream concurrently — a pair costs ~one N=512 stream instead of
        # two). Keeping group == pair gives both members IDENTICAL deps
        # (same st tile): the Tile scheduler reorders ready work around
        # stalled instructions, and members with different deps get split by
        # hoisted MM2s, which serializes the pack. st tiles are [128,1024]
        # (2 PSUM banks; stA+stB = 4) leaving 4 banks for pbp.
        for pair in range(PAIRS):
            for qcb in range(NQB):
                q0 = qcb * QB
                for kc in range(KC):
                    tag = "stA" if kc % 2 == 0 else "stB"
                    st = stps.tile([P, 2 * QB], f32, tag=tag, name="st")
                    for j in (0, 1):
                        nc.tensor.matmul(
                            st[:, j * QB:(j + 1) * QB],
                            KT[pair][j * DK:(j + 1) * DK, kc * P:(kc + 1) * P],
                            QT[pair][j * DK:(j + 1) * DK, q0:q0 + QB],
                            start=True, stop=True,
                            tile_position=(j * DK, 0))
                    pt = ptp.tile([P, 2 * QB], bf16, tag="pt", name="pt")
                    nc.scalar.activation(pt[:], st[:], Exp, scale=0.125)
                    for j in (0, 1):
                        pending.append(
                            (pair, qcb, pt, j, j, kc,
                             2 * kc + j + 1 == 2 * KC))
                    gc_box[0] += 1
                    # boundary: all full-array PE work goes here so it never
                    # lands inside a row-tiled MM1 pack.
                    # flushes first: an MM2's 65-col weight load refills the
                    # weight buffer ~2x faster after a pack than a
                    # projection's 128-col one.
                    for _ in range(2):
                        if len(pending) > TDU and pending[0][5] < v_done[0]:
                            flush_mm2_unit()
                    if iter0_items:
                        for _ in range(2):
                            if iter0_items:
                                for th in iter0_items.pop(0):
                                    th()
                    else:
                        pump(4 * pair + qcb)
        while pending:
            flush_mm2_unit()
        while bg_items:
            for th in bg_items.pop(0)[1]:
                th()
        # Held chunks first: their HOUT is long ready, so the scheduler can
        # run them under the last iteration's normalize-chain latency.
        for oq, ok in held_items:
            outproj_qk(oq, ok)
        while oproj_items:
            _, oq, ok = oproj_items.pop(0)
            outproj_qk(oq, ok)


def build_program():
    """Build + compile the single-core SPMD Bass program. Cached per process."""
    if "nc" in _PROGRAM_CACHE:
        return _PROGRAM_CACHE["nc"]
    import concourse.bacc as bacc
    import concourse.tile as tile
    import concourse.mybir as mybir

    bf16 = mybir.dt.float16  # fp16: same PE rate as bf16, 4x finer mantissa
    f32 = mybir.dt.float32
    nc = bacc.Bacc("TRN2", target_bir_lowering=False, debug=False)
    aps = {
        "qT": nc.dram_tensor("qT", [D, S], bf16, kind="ExternalInput").ap(),
        "kT": nc.dram_tensor("kT", [D, S], bf16, kind="ExternalInput").ap(),
        "vT": nc.dram_tensor("vT", [D, S], bf16, kind="ExternalInput").ap(),
        "wq": nc.dram_tensor("wq", [D, COLS], bf16, kind="ExternalInput").ap(),
        "wo": nc.dram_tensor("wo", [COLS, D], bf16, kind="ExternalInput").ap(),
        "out": nc.dram_tensor("out", [S, D], f32, kind="ExternalOutput").ap(),
    }
    with tile.TileContext(nc) as tc:
        _emit_kernel(tc, aps)
    nc.compile()
    _PROGRAM_CACHE["nc"] = nc
    return nc


def make_in_maps(q, k, v, Wq, Wo):
    """Host-side sharding: core c -> batch c//2, head-half c%2."""
    q = np.asarray(q, dtype=np.float32)
    k = np.asarray(k, dtype=np.float32)
    v = np.asarray(v, dtype=np.float32)
    Wq = np.asarray(Wq, dtype=np.float32)
    Wo = np.asarray(Wo, dtype=np.float32)
    WqT = np.ascontiguousarray(Wq.T)   # (in D, out D)
    WoT = np.ascontiguousarray(Wo.T)   # (in D, out D)
    in_maps = []
    for c in range(N_CORES):
        b, half = divmod(c, 2)
        cols = slice(half * COLS, (half + 1) * COLS)
        in_maps.append({
            "qT": np.ascontiguousarray(q[b].T).astype(F16),
            "kT": np.ascontiguousarray(k[b].T).astype(F16),
            "vT": np.ascontiguousarray(v[b].T).astype(F16),
            "wq": np.ascontiguousarray(WqT[:, cols]).astype(F16),
            "wo": np.ascontiguousarray(WoT[cols, :]).astype(F16),
        })
    return in_maps


def run_cores(in_maps, trace=False, trace_cores=None):
    from concourse.bass_utils import run_bass_kernel_spmd
    nc = build_program()
    return run_bass_kernel_spmd(
        nc, in_maps, core_ids=list(range(N_CORES)),
        trace=trace, trace_cores=trace_cores)


def kernel(q, k, v, Wq, Wo):
    in_maps = make_in_maps(q, k, v, Wq, Wo)
    res = run_cores(in_maps)
    B = 4
    out = np.zeros((B, S, D), dtype=np.float32)
    for c in range(N_CORES):
        out[c // 2] += res.results[c]["out"]
    return out

